# revision 20
# baseline (speedup 1.0000x reference)
"""Trainium2 Bass kernel for nn_Model1 (lag-weighted long-run covariance + MLP).

Math: the 129-lag weighted covariance collapses algebraically:
    sum_l w_l * (Xc @ Y_l.T) = Xc @ (sum_l w_l Y_l).T
where Y_l is the (masked) l-shift of Xc.  So cov = (Xc @ P.T + N @ Xc.T)/d
with P, N two 65-tap causal FIR filters of Xc.  Centering is pushed through
the filters as rank-1 corrections so all GEMMs run on UNCENTERED X:
    cov.T * d = U.T - m (x) alpha - beta (x) m
with U = X@P0.T + N0@X.T (P0,N0 = filters of raw X), m = row means, and
alpha/beta derived from 5 reduction vectors (r,a,c,p,q) that are computed
as extra tiny matmuls and ride along in the AllReduce.

Distribution (8 cores):
  - cov stage: shard time axis (512 cols/core), one AllReduce of [261,256] f32
  - MLP: tensor-parallel over hidden (512/core), AllGather (bf16) between
    fc1->fc2 and fc2->fc3; fc3 emits batch-major so proj shards the output
    columns; final gather is a host-side concat.

Scheduling (the perf-critical part):
  - DMA rings: sync + gpsimd carry ONLY bulk weight/X streams (deadline
    order: w1 -> w2 -> w3 -> pj); scalar carries ONLY latency-critical
    small transfers (AR staging/readback, g1/g2 stores); vector+scalar
    share the gathered-activation readbacks.  This keeps the cov AllReduce
    staging from queuing behind megabytes of weights (the baseline's main
    stall: AR fired at 85us instead of ~15us).
  - Collective triggers stay on gpsimd (only engine with collective_compute)
    AFTER all its bulk dma descriptor-gens.
  - fc3 loops k-outer/ii-inner so consecutive matmuls alternate PSUM banks.
  - a1f/a2f gathered readbacks are 8 x 256KB HWDGE transfers (vector+scalar),
    not serial gpsimd DIRECT2D copies.
  - GELU table preloaded with a dummy activation at t=0.
All heavy GEMMs use bf16 operands with fp32 PSUM accumulation.
"""
import math
import numpy as np
import ml_dtypes

NCORES = 8
Q = 64
NN = 256          # n (batch/rows of X)
DD = 4096         # d (time axis)
HID = 4096
Y0 = 512
HSH = HID // NCORES    # 512 hidden shard per core
NBLK = DD // 128       # 32 time blocks
BPC = NBLK // NCORES   # 4 blocks per core
KB1 = (DD + DD) // 128  # 64 fc1 contraction blocks
KB = HID // 128         # 32 fc2/fc3 contraction blocks
XHS = 264              # xh per-block col stride (256 data + 1 ones + pad)

bf16 = ml_dtypes.bfloat16

_CACHE = {}


# ----------------------------------------------------------------------------
# host-side weight-only precompute
# ----------------------------------------------------------------------------
def _erf(x):
    return np.vectorize(math.erf, otypes=[np.float64])(x)


def _gelu64(x):
    return 0.5 * x * (1.0 + _erf(x / np.sqrt(2.0)))


def _filters(inputs):
    f64 = lambda k: np.asarray(inputs[k], np.float64)
    lags = np.arange(-Q, Q + 1, dtype=np.float64)[:, None]
    h = _gelu64(lags @ f64("wn_w1") + f64("wn_b1"))
    w = (h @ f64("wn_w2") + f64("wn_b2"))[:, 0]
    wp = w[Q:]                                    # l = 0..Q
    wnv = np.concatenate([[0.0], w[:Q][::-1]])    # wnv[l] = w[Q-l], l=1..Q
    v = np.arange(128)[:, None]
    u = np.arange(128)[None, :]
    dvu = v - u
    d2 = dvu + 128
    A0 = np.where((dvu >= 0) & (dvu <= Q), wp[np.clip(dvu, 0, Q)], 0.0)
    A1 = np.where((d2 >= 0) & (d2 <= Q), wp[np.clip(d2, 0, Q)], 0.0)
    B0 = np.where((dvu >= 1) & (dvu <= Q), wnv[np.clip(dvu, 0, Q)], 0.0)
    B1 = np.where((d2 >= 1) & (d2 <= Q), wnv[np.clip(d2, 0, Q)], 0.0)
    t = np.arange(DD)
    lim = np.minimum(Q, DD - 1 - t)
    g_p = np.cumsum(wp)[lim]
    g_n = np.cumsum(wnv)[lim]
    gamma = float(g_p.sum() + g_n.sum())
    return A0, A1, B0, B1, g_p, g_n, gamma


# ----------------------------------------------------------------------------
# bass program
# ----------------------------------------------------------------------------
def build(gamma: float, use_gelu=True):
    import concourse.bacc as bacc
    import concourse.tile as tile
    import concourse.mybir as mybir

    dt32 = mybir.dt.float32
    dt16 = mybir.dt.bfloat16
    GELU = (mybir.ActivationFunctionType.Gelu if use_gelu
            else mybir.ActivationFunctionType.Identity)

    nc = bacc.Bacc("TRN2", target_bir_lowering=False, debug=False,
                   num_devices=NCORES)
    mm = nc.tensor.matmul
    RG = [list(range(NCORES))]

    # ---- I/O ----
    xt_d = nc.dram_tensor("xt", [128, NBLK * 256], dt16, kind="ExternalInput").ap()
    xh_d = nc.dram_tensor("xh", [128, 5 * XHS], dt16, kind="ExternalInput").ap()
    tp_d = nc.dram_tensor("tp", [128, 512], dt16, kind="ExternalInput").ap()
    aux_d = nc.dram_tensor("aux", [128, BPC * 3], dt16, kind="ExternalInput").ap()
    w1_d = nc.dram_tensor("w1", [128, KB1 * 512], dt16, kind="ExternalInput").ap()
    w2_d = nc.dram_tensor("w2", [128, KB * 512], dt16, kind="ExternalInput").ap()
    w3_d = nc.dram_tensor("w3", [128, KB * 512], dt16, kind="ExternalInput").ap()
    w2l_d = nc.dram_tensor("w2l", [128, 4 * 512], dt16, kind="ExternalInput").ap()
    w3l_d = nc.dram_tensor("w3l", [128, 4 * 512], dt16, kind="ExternalInput").ap()
    pj_d = nc.dram_tensor("pj", [128, 2 * 512], dt16, kind="ExternalInput").ap()
    b1_d = nc.dram_tensor("b1", [128, 4], dt32, kind="ExternalInput").ap()
    b2_d = nc.dram_tensor("b2", [128, 4], dt32, kind="ExternalInput").ap()
    b3_d = nc.dram_tensor("b3", [1, 512], dt16, kind="ExternalInput").ap()
    out_d = nc.dram_tensor("out", [Y0, HSH], dt32, kind="ExternalOutput").ap()

    with tile.TileContext(nc) as tc:
        with (
            tc.tile_pool(name="cst", bufs=1) as cst,
            tc.tile_pool(name="pn", bufs=3) as pnp,
            tc.tile_pool(name="osb", bufs=2) as osp,
            tc.tile_pool(name="psA", bufs=1, space="PSUM") as psA,
            tc.tile_pool(name="psB", bufs=2, space="PSUM") as psB,
            tc.tile_pool(name="dram", bufs=1, space="DRAM") as drp,
        ):
            # ================= t0: tiny latency-critical loads =============
            xh_t = cst.tile([128, 5 * XHS], dt16, tag="xh")
            nc.sync.dma_start(xh_t, xh_d)
            tp_t = cst.tile([128, 512], dt16, tag="tp")
            nc.sync.dma_start(tp_t, tp_d)
            aux_t = cst.tile([128, BPC * 3], dt16, tag="aux")
            nc.sync.dma_start(aux_t, aux_d)
            b1_t = cst.tile([128, 4], dt32, tag="b1")
            nc.scalar.dma_start(b1_t, b1_d)
            b2_t = cst.tile([128, 4], dt32, tag="b2")
            nc.scalar.dma_start(b2_t, b2_d)
            b3_t = cst.tile([1, 512], dt16, tag="b3")
            nc.scalar.dma_start(b3_t, b3_d)
            ones_t = cst.tile([128, 1], dt16, tag="ones")
            nc.vector.memset(ones_t, 1.0)
            onesr_t = cst.tile([1, 128], dt16, tag="onesr")
            nc.vector.memset(onesr_t, 1.0)
            # preload the GELU activation table off the critical path
            dum_t = cst.tile([1, 1], dt16, tag="dum")
            nc.scalar.activation(dum_t, ones_t[0:1, 0:1],
                                 GELU, bias=b1_t[0:1, 0:1])

            # ================= bulk streams (deadline order) ===============
            # gpsimd ring: xt, then its half of w1/w2/w3
            xt_t = cst.tile([128, NBLK * 256], dt16, tag="xt")
            for sp in range(4):
                w = NBLK * 256 // 4
                nc.sync.dma_start(xt_t[:, w * sp: w * (sp + 1)],
                                  xt_d[:, w * sp: w * (sp + 1)])

            # w1 fully resident: its stream drains at wire speed instead of
            # being paced by fc1's matmul consumption through a small pool.
            w1R = cst.tile([128, KB1 * 512], dt16, tag="w1R")
            for sp in range(8):
                w = KB1 * 512 // 8
                nc.sync.dma_start(w1R[:, w * sp: w * (sp + 1)],
                                  w1_d[:, w * sp: w * (sp + 1)])

            # DRAM bounce buffers for the collectives
            arA_i = drp.tile([261, NN], dt32, tag="arA_i")
            arA_o = drp.tile([261, NN], dt32, tag="arA_o", addr_space="Shared")
            g1_i = drp.tile([128, 1024], dt16, tag="g1_i")
            g1_o = drp.tile([1024, 1024], dt16, tag="g1_o", addr_space="Shared")
            g2_i = drp.tile([128, 1024], dt16, tag="g2_i")
            g2_o = drp.tile([1024, 1024], dt16, tag="g2_o", addr_space="Shared")

            # ================= stage 1: cov partials =======================
            # u_ps[ic]: UT chunk [a in 128*ic.., b]; racp: rac rows 0:3 and
            # p at partition 32 (one bank); q its own bank.
            u_ps = [psA.tile([128, 256], dt32, tag=f"acc{ic}", name=f"u{ic}")
                    for ic in range(2)]
            racp_ps = psA.tile([33, 256], dt32, tag="acc2", name="racp_ps")
            q_ps = psA.tile([1, 256], dt32, tag="acc3", name="q_ps")
            for bl in range(BPC):
                xb = xh_t[:, XHS * bl: XHS * bl + 256]
                xb1 = xh_t[:, XHS * (bl + 1): XHS * (bl + 1) + 256]
                first, last = bl == 0, bl == BPC - 1
                pt_ps = psB.tile([128, 256], dt32, tag="rot", name="pt_ps")
                mm(pt_ps, tp_t[:, 0:128], xb, start=True, stop=False)
                mm(pt_ps, tp_t[:, 128:256], xb1, start=False, stop=True)
                pt_sb = pnp.tile([128, 256], dt16, tag="ptsb", name="pt_sb")
                nc.vector.tensor_copy(pt_sb, pt_ps)
                nt_ps = psB.tile([128, 256], dt32, tag="rot", name="nt_ps")
                mm(nt_ps, tp_t[:, 256:384], xb, start=True, stop=False)
                mm(nt_ps, tp_t[:, 384:512], xb1, start=False, stop=True)
                nt_sb = pnp.tile([128, 256], dt16, tag="ntsb", name="nt_sb")
                nc.vector.tensor_copy(nt_sb, nt_ps)
                for ic in range(2):
                    xbc = xh_t[:, XHS * bl + 128 * ic: XHS * bl + 128 * ic + 128]
                    mm(u_ps[ic], pt_sb[:, 128 * ic:128 * ic + 128], xb,
                       start=first, stop=False)
                    mm(u_ps[ic], xbc, nt_sb, start=False, stop=last)
                # reduction vectors: rac rows 0:3 (start clears the bank),
                # p at partition 32 rides the same bank with start=False.
                mm(racp_ps[0:3, :], aux_t[:, 3 * bl:3 * bl + 3], xb,
                   start=first, stop=False, skip_group_check=True)
                mm(racp_ps[32:33, :], ones_t, pt_sb,
                   start=False, stop=last, skip_group_check=True)
                mm(q_ps, ones_t, nt_sb, start=first, stop=last)

            # ---- stage AR input (scalar ring = empty -> lands immediately)
            stgs = []
            for ic in range(2):
                stg = cst.tile([128, 256], dt32, tag=f"stg{ic}", name=f"stg{ic}")
                nc.vector.tensor_scalar_mul(stg, u_ps[ic], 1.0 / DD)
                nc.scalar.dma_start(arA_i[128 * ic:128 * ic + 128, :], stg)
                stgs.append(stg)
            rpq_sb = cst.tile([65, 256], dt32, tag="rpq_sb")
            nc.vector.tensor_copy(rpq_sb[0:3, :], racp_ps[0:3, :])
            nc.vector.tensor_copy(rpq_sb[32:33, :], racp_ps[32:33, :])
            nc.vector.tensor_copy(rpq_sb[64:65, :], q_ps)
            nc.scalar.dma_start(arA_i[256:259, :], rpq_sb[0:3, :])
            nc.scalar.dma_start(arA_i[259:260, :], rpq_sb[32:33, :])
            nc.scalar.dma_start(arA_i[260:261, :], rpq_sb[64:65, :])
            nc.gpsimd.collective_compute(
                "AllReduce", mybir.AluOpType.add, replica_groups=RG,
                ins=[arA_i.opt()], outs=[arA_o.opt()])

            # ================= fc1 X-half (overlaps AllReduce) =============
            f1_ps = [psA.tile([128, 256], dt32, tag=f"acc{hh}", name=f"f1_{hh}")
                     for hh in range(4)]
            for k in range(NBLK):
                for hh in range(4):
                    mm(f1_ps[hh],
                       w1R[:, 512 * k + 128 * hh: 512 * k + 128 * hh + 128],
                       xt_t[:, 256 * k:256 * k + 256],
                       start=(k == 0), stop=False)

            # ================= G^T = X @ W1c (also in AR shadow) ===========
            g_ps = [psA.tile([128, 512], dt32, tag=f"acc{4 + ib}", name=f"g_{ib}")
                    for ib in range(2)]
            for k in range(NBLK):
                for ib in range(2):
                    mm(g_ps[ib],
                       xt_t[:, 256 * k + 128 * ib: 256 * k + 128 * ib + 128],
                       w1R[:, 512 * (NBLK + k): 512 * (NBLK + k) + 512],
                       start=(k == 0), stop=(k == NBLK - 1))

            # ================= remaining bulk (w2, w3, pj) =================
            # w2l/w3l early (local-shard matmuls run in the AllGather
            # shadows).  w2R/w3R/pj DMAs are emitted later behind gate-reader
            # ops with REAL data deps so the scheduler cannot hoist them into
            # the shared HWDGE FIFO ahead of the AR staging/readback.
            w2l_t = cst.tile([128, 4 * 512], dt16, tag="w2l")
            nc.sync.dma_start(w2l_t, w2l_d)
            w3l_t = cst.tile([128, 4 * 512], dt16, tag="w3l")
            nc.sync.dma_start(w3l_t, w3l_d)
            w2R = cst.tile([128, KB * 512], dt16, tag="w2R")
            w3R = cst.tile([128, KB * 512], dt16, tag="w3R")
            pj_t = cst.tile([128, 2 * 512], dt16, tag="pj")
            nc.vector.memset(w2R[0:1, 0:KB * 512:KB * 64], 0.0)
            nc.vector.memset(w3R[0:1, 0:KB * 512:KB * 64], 0.0)
            nc.vector.memset(pj_t[0:1, 0:1024:512], 0.0)
            gsc1 = cst.tile([1, 8], dt16, tag="gsc1")
            gsc2 = cst.tile([1, 8], dt16, tag="gsc2")
            gsc3 = cst.tile([1, 2], dt16, tag="gsc3")

            # ================= corrections + covT ==========================
            ured = [cst.tile([128, 256], dt32, tag=f"stg{ic}", name=f"ured{ic}")
                    for ic in range(2)]
            for ic in range(2):
                nc.scalar.dma_start(ured[ic], arA_o[128 * ic:128 * ic + 128, :])
            rows5 = cst.tile([1, 5 * 256], dt32, tag="rows5")
            nc.scalar.dma_start(rows5, arA_o[256:261, :])
            r_row, a_row, c_row = (rows5[:, 0:256], rows5[:, 256:512],
                                   rows5[:, 512:768])
            p_row, q_row = rows5[:, 768:1024], rows5[:, 1024:1280]
            # staged U (hence ured) is pre-divided by D, so alpha/beta are
            # divided by D here too; m = r/D.
            m16 = cst.tile([1, NN], dt16, tag="m16")
            nc.vector.tensor_scalar_mul(m16, r_row, 1.0 / DD)
            t_aq = cst.tile([1, NN], dt32, tag="t_aq")
            nc.vector.tensor_add(t_aq, a_row, q_row)
            al32 = cst.tile([1, NN], dt32, tag="al32")
            nc.vector.tensor_scalar_mul(al32, t_aq, 1.0 / DD)
            gm32 = cst.tile([1, NN], dt32, tag="gm32")
            nc.vector.tensor_scalar_mul(gm32, r_row, gamma / (DD * DD))
            al16 = cst.tile([1, NN], dt16, tag="al16")
            nc.vector.tensor_sub(al16, al32, gm32)
            t_pc = cst.tile([1, NN], dt32, tag="t_pc")
            nc.vector.tensor_add(t_pc, p_row, c_row)
            be16 = cst.tile([1, NN], dt16, tag="be16")
            nc.vector.tensor_scalar_mul(be16, t_pc, 1.0 / DD)

            # gate: the FIRST HALF of w2R WAR-waits this reader, entering
            # the DMA FIFO once the AR result (m16) is back -- early enough
            # to feed fc2's first 16 k-blocks, small enough not to delay the
            # g1_i store / AG1 trigger by much.
            nc.vector.tensor_add(gsc1, m16[0:1, 0:8],
                                 w2R[0:1, 0:KB * 512:KB * 64])
            for sp in range(4):
                w = KB * 512 // 8
                nc.sync.dma_start(w2R[:, w * sp: w * (sp + 1)],
                                  w2_d[:, w * sp: w * (sp + 1)])

            covt = cst.tile([128, 2 * 256], dt16, tag="covt")
            for ic in range(2):
                corr = psB.tile([128, 256], dt32, tag="rot", name="corr")
                mm(corr, m16[:, 128 * ic:128 * ic + 128], al16,
                   start=True, stop=False)
                mm(corr, be16[:, 128 * ic:128 * ic + 128], m16,
                   start=False, stop=True)
                nc.vector.tensor_sub(covt[:, 256 * ic:256 * ic + 256],
                                     ured[ic], corr)

            # ================= fc1 cov contribution + gelu =================
            gT = cst.tile([128, 2 * 512], dt16, tag="gT")
            for ib in range(2):
                nc.vector.tensor_copy(gT[:, 512 * ib:512 * ib + 512], g_ps[ib])
            for hh in range(4):
                for ib in range(2):
                    mm(f1_ps[hh], gT[:, 512 * ib + 128 * hh: 512 * ib + 128 * hh + 128],
                       covt[:, 256 * ib:256 * ib + 256],
                       start=False, stop=(ib == 1))
            a1loc = cst.tile([128, 4 * 256], dt16, tag="a1loc")
            for hh in range(4):
                nc.scalar.activation(a1loc[:, 256 * hh:256 * hh + 256],
                                     f1_ps[hh], GELU, bias=b1_t[:, hh:hh + 1])
            nc.scalar.dma_start(g1_i, a1loc)

            # ================= AllGather a1 + fc2 ==========================
            nc.gpsimd.collective_compute(
                "AllGather", mybir.AluOpType.bypass, replica_groups=RG,
                ins=[g1_i.opt()], outs=[g1_o.opt()])
            f2_ps = [psA.tile([128, 256], dt32, tag=f"acc{hh}", name=f"f2_{hh}")
                     for hh in range(4)]
            # local chunk contribution (runs during the AllGather); the
            # corresponding k-blocks of w2R are zeroed host-side
            for j in range(BPC):
                for hh in range(4):
                    mm(f2_ps[hh],
                       w2l_t[:, 512 * j + 128 * hh: 512 * j + 128 * hh + 128],
                       a1loc[:, 256 * j:256 * j + 256],
                       start=(j == 0), stop=False)
            a1f = cst.tile([128, NBLK * 256], dt16, tag="a1f")
            for r in range(NCORES):
                eng = nc.sync if r % 2 == 0 else nc.scalar
                eng.dma_start(a1f[:, 1024 * r:1024 * r + 1024],
                              g1_o[128 * r:128 * r + 128, :])
            # second half of w2R + w3R/pj gated on the first gathered a1
            # chunk: they enter the DMA FIFO after the AG1 readback, clearing
            # the wire for the AG1 critical path while landing just ahead of
            # fc2's second half and fc3.
            gsc4 = cst.tile([1, 4], dt16, tag="gsc4")
            nc.vector.tensor_add(gsc4, a1f[0:1, 24:28],
                                 w2R[0:1, KB * 256:KB * 512:KB * 64])
            for sp in range(4, 8):
                w = KB * 512 // 8
                nc.sync.dma_start(w2R[:, w * sp: w * (sp + 1)],
                                  w2_d[:, w * sp: w * (sp + 1)])
            nc.vector.tensor_add(gsc2, a1f[0:1, 0:8],
                                 w3R[0:1, 0:KB * 512:KB * 64])
            nc.vector.tensor_add(gsc3, a1f[0:1, 8:10],
                                 pj_t[0:1, 0:1024:512])
            for sp in range(8):
                w = KB * 512 // 8
                nc.sync.dma_start(w3R[:, w * sp: w * (sp + 1)],
                                  w3_d[:, w * sp: w * (sp + 1)])
            nc.sync.dma_start(pj_t, pj_d)
            for k in range(KB):
                for hh in range(4):
                    mm(f2_ps[hh],
                       w2R[:, 512 * k + 128 * hh: 512 * k + 128 * hh + 128],
                       a1f[:, 256 * k:256 * k + 256],
                       start=False, stop=(k == KB - 1))
            a2loc = cst.tile([128, 4 * 256], dt16, tag="a2loc")
            for hh in range(4):
                nc.scalar.activation(a2loc[:, 256 * hh:256 * hh + 256],
                                     f2_ps[hh], GELU, bias=b2_t[:, hh:hh + 1])
            nc.scalar.dma_start(g2_i, a2loc)

            # ================= AllGather a2 + fc3 ==========================
            nc.gpsimd.collective_compute(
                "AllGather", mybir.AluOpType.bypass, replica_groups=RG,
                ins=[g2_i.opt()], outs=[g2_o.opt()])
            f3_ps = [psA.tile([128, 512], dt32, tag=f"acc{4 + ii}", name=f"f3_{ii}")
                     for ii in range(2)]
            for j in range(BPC):   # local chunk, runs during the AllGather
                for ii in range(2):
                    mm(f3_ps[ii],
                       a2loc[:, 256 * j + 128 * ii: 256 * j + 128 * ii + 128],
                       w3l_t[:, 512 * j: 512 * j + 512],
                       start=(j == 0), stop=False)
            a2f = cst.tile([128, NBLK * 256], dt16, tag="a1f", name="a2f")
            for r in range(NCORES):
                eng = nc.sync if r % 2 == 0 else nc.scalar
                eng.dma_start(a2f[:, 1024 * r:1024 * r + 1024],
                              g2_o[128 * r:128 * r + 128, :])
            # k outer / ii inner: consecutive matmuls alternate PSUM banks
            for k in range(KB):
                for ii in range(2):
                    mm(f3_ps[ii],
                       a2f[:, 256 * k + 128 * ii: 256 * k + 128 * ii + 128],
                       w3R[:, 512 * k: 512 * k + 512],
                       start=False, stop=False)
            o3_t = cst.tile([128, 2 * 512], dt16, tag="o3")
            for ii in range(2):
                mm(f3_ps[ii], onesr_t, b3_t, start=False, stop=True)
                nc.vector.tensor_copy(o3_t[:, 512 * ii:512 * ii + 512], f3_ps[ii])

            # ================= proj ========================================
            for pp in range(4):
                po = psB.tile([128, 512], dt32, tag="rot", name="po")
                for ii in range(2):
                    mm(po, pj_t[:, 512 * ii + 128 * pp: 512 * ii + 128 * pp + 128],
                       o3_t[:, 512 * ii:512 * ii + 512],
                       start=(ii == 0), stop=(ii == 1))
                osb = osp.tile([128, 512], dt32, tag="osb", name=f"osb{pp}")
                nc.vector.tensor_copy(osb, po)
                eng = nc.scalar if pp % 2 == 0 else nc.sync
                eng.dma_start(out_d[128 * pp:128 * pp + 128, :], osb)

    nc.compile()
    return nc


# ----------------------------------------------------------------------------
# host-side sharding / packing
# ----------------------------------------------------------------------------
def prep_in_maps(inputs):
    X = np.asarray(inputs["X"], np.float32)
    A0, A1, B0, B1, g_p, g_n, gamma = _filters(inputs)

    XT = np.ascontiguousarray(X.T)                      # [D, N]
    xt = XT.reshape(NBLK, 128, NN).transpose(1, 0, 2).reshape(128, NBLK * 256)
    xt = xt.astype(bf16)
    tp = np.concatenate([A0, A1, B0, B1], axis=1).astype(bf16)
    pjT = np.asarray(inputs["proj"], np.float64).T      # [256, 512]
    pj = pjT.reshape(2, 128, 512).transpose(1, 0, 2).reshape(128, 1024).astype(bf16)

    f64 = lambda k: np.asarray(inputs[k], np.float64)
    fc_wT = {1: f64("fc1_w").T, 2: f64("fc2_w").T, 3: f64("fc3_w").T}

    XTz = np.concatenate([XT, np.zeros((128, NN), np.float32)], axis=0)

    in_maps = []
    for c in range(NCORES):
        # xh: 5 blocks (4 local + halo), stride 264, ones col at 256
        xh = np.zeros((128, 5 * XHS), np.float32)
        for bl in range(5):
            gb = 4 * c + bl
            xh[:, XHS * bl: XHS * bl + 256] = XTz[128 * gb:128 * gb + 128]
            xh[:, XHS * bl + 256] = 1.0
        aux = np.zeros((128, BPC * 3), np.float32)
        for bl in range(BPC):
            gb = 4 * c + bl
            aux[:, 3 * bl + 0] = 1.0
            aux[:, 3 * bl + 1] = g_p[128 * gb:128 * gb + 128]
            aux[:, 3 * bl + 2] = g_n[128 * gb:128 * gb + 128]
        hs = slice(HSH * c, HSH * (c + 1))
        w1 = fc_wT[1][:, hs].reshape(KB1, 128, HSH).transpose(1, 0, 2) \
            .reshape(128, KB1 * HSH).astype(bf16)
        w2full = fc_wT[2][:, hs].reshape(KB, 128, HSH)
        w3full = fc_wT[3][:, hs].reshape(KB, 128, HSH)
        lb = slice(BPC * c, BPC * (c + 1))       # this core's local k-blocks
        w2l = w2full[lb].transpose(1, 0, 2).reshape(128, BPC * HSH).astype(bf16)
        w3l = w3full[lb].transpose(1, 0, 2).reshape(128, BPC * HSH).astype(bf16)
        w2full = w2full.copy(); w2full[lb] = 0.0
        w3full = w3full.copy(); w3full[lb] = 0.0
        w2 = w2full.transpose(1, 0, 2).reshape(128, KB * HSH).astype(bf16)
        w3 = w3full.transpose(1, 0, 2).reshape(128, KB * HSH).astype(bf16)
        b1 = f64("fc1_b")[hs].reshape(4, 128).T.astype(np.float32)
        b2 = f64("fc2_b")[hs].reshape(4, 128).T.astype(np.float32)
        b3 = f64("fc3_b")[hs].reshape(1, HSH).astype(bf16)
        in_maps.append({
            "xt": xt, "xh": xh.astype(bf16), "tp": tp,
            "aux": aux.astype(bf16), "w1": w1, "w2": w2, "w3": w3,
            "w2l": w2l, "w3l": w3l,
            "pj": pj, "b1": b1, "b2": b2, "b3": b3,
        })
    return in_maps, gamma


def run(inputs, trace=False, **kw):
    in_maps, gamma = prep_in_maps(inputs)
    key = ("nc", float(gamma))
    if key not in _CACHE:
        _CACHE[key] = build(gamma)
    nc = _CACHE[key]
    from concourse import bass_utils
    res = bass_utils.run_bass_kernel_spmd(nc, in_maps,
                                          core_ids=list(range(NCORES)),
                                          trace=trace, **kw)
    out = np.concatenate([res.results[c]["out"] for c in range(NCORES)], axis=1)
    return out.astype(np.float32), res


def kernel(**inputs) -> np.ndarray:
    out, _ = run(inputs)
    return out


if __name__ == "__main__":
    data = np.load("inputs.npz")
    inputs = {k: data[k] for k in data.files}
    expected = np.load("expected.npy")
    out = kernel(**inputs)
    scale = np.abs(expected).max()
    err = np.abs(out - expected).max() / scale
    print(f"Relative error: {err:.3e}")


# revision 21
# speedup vs baseline: 1.0024x; 1.0024x over previous
"""Trainium2 Bass kernel for nn_Model1 (lag-weighted long-run covariance + MLP).

Math: the 129-lag weighted covariance collapses algebraically:
    sum_l w_l * (Xc @ Y_l.T) = Xc @ (sum_l w_l Y_l).T
where Y_l is the (masked) l-shift of Xc.  So cov = (Xc @ P.T + N @ Xc.T)/d
with P, N two 65-tap causal FIR filters of Xc.  Centering is pushed through
the filters as rank-1 corrections so all GEMMs run on UNCENTERED X:
    cov.T * d = U.T - m (x) alpha - beta (x) m
with U = X@P0.T + N0@X.T (P0,N0 = filters of raw X), m = row means, and
alpha/beta derived from 5 reduction vectors (r,a,c,p,q) that are computed
as extra tiny matmuls and ride along in the AllReduce.

Distribution (8 cores):
  - cov stage: shard time axis (512 cols/core), one AllReduce of [261,256] f32
  - MLP: tensor-parallel over hidden (512/core), AllGather (bf16) between
    fc1->fc2 and fc2->fc3; fc3 emits batch-major so proj shards the output
    columns; final gather is a host-side concat.

Scheduling (the perf-critical part):
  - DMA rings: sync + gpsimd carry ONLY bulk weight/X streams (deadline
    order: w1 -> w2 -> w3 -> pj); scalar carries ONLY latency-critical
    small transfers (AR staging/readback, g1/g2 stores); vector+scalar
    share the gathered-activation readbacks.  This keeps the cov AllReduce
    staging from queuing behind megabytes of weights (the baseline's main
    stall: AR fired at 85us instead of ~15us).
  - Collective triggers stay on gpsimd (only engine with collective_compute)
    AFTER all its bulk dma descriptor-gens.
  - fc3 loops k-outer/ii-inner so consecutive matmuls alternate PSUM banks.
  - a1f/a2f gathered readbacks are 8 x 256KB HWDGE transfers (vector+scalar),
    not serial gpsimd DIRECT2D copies.
  - GELU table preloaded with a dummy activation at t=0.
All heavy GEMMs use bf16 operands with fp32 PSUM accumulation.
"""
import math
import numpy as np
import ml_dtypes

NCORES = 8
Q = 64
NN = 256          # n (batch/rows of X)
DD = 4096         # d (time axis)
HID = 4096
Y0 = 512
HSH = HID // NCORES    # 512 hidden shard per core
NBLK = DD // 128       # 32 time blocks
BPC = NBLK // NCORES   # 4 blocks per core
KB1 = (DD + DD) // 128  # 64 fc1 contraction blocks
KB = HID // 128         # 32 fc2/fc3 contraction blocks
XHS = 264              # xh per-block col stride (256 data + 1 ones + pad)

bf16 = ml_dtypes.bfloat16

_CACHE = {}


# ----------------------------------------------------------------------------
# host-side weight-only precompute
# ----------------------------------------------------------------------------
def _erf(x):
    return np.vectorize(math.erf, otypes=[np.float64])(x)


def _gelu64(x):
    return 0.5 * x * (1.0 + _erf(x / np.sqrt(2.0)))


def _filters(inputs):
    f64 = lambda k: np.asarray(inputs[k], np.float64)
    lags = np.arange(-Q, Q + 1, dtype=np.float64)[:, None]
    h = _gelu64(lags @ f64("wn_w1") + f64("wn_b1"))
    w = (h @ f64("wn_w2") + f64("wn_b2"))[:, 0]
    wp = w[Q:]                                    # l = 0..Q
    wnv = np.concatenate([[0.0], w[:Q][::-1]])    # wnv[l] = w[Q-l], l=1..Q
    v = np.arange(128)[:, None]
    u = np.arange(128)[None, :]
    dvu = v - u
    d2 = dvu + 128
    A0 = np.where((dvu >= 0) & (dvu <= Q), wp[np.clip(dvu, 0, Q)], 0.0)
    A1 = np.where((d2 >= 0) & (d2 <= Q), wp[np.clip(d2, 0, Q)], 0.0)
    B0 = np.where((dvu >= 1) & (dvu <= Q), wnv[np.clip(dvu, 0, Q)], 0.0)
    B1 = np.where((d2 >= 1) & (d2 <= Q), wnv[np.clip(d2, 0, Q)], 0.0)
    t = np.arange(DD)
    lim = np.minimum(Q, DD - 1 - t)
    g_p = np.cumsum(wp)[lim]
    g_n = np.cumsum(wnv)[lim]
    gamma = float(g_p.sum() + g_n.sum())
    return A0, A1, B0, B1, g_p, g_n, gamma


# ----------------------------------------------------------------------------
# bass program
# ----------------------------------------------------------------------------
def build(gamma: float, use_gelu=True):
    import concourse.bacc as bacc
    import concourse.tile as tile
    import concourse.mybir as mybir

    dt32 = mybir.dt.float32
    dt16 = mybir.dt.bfloat16
    GELU = (mybir.ActivationFunctionType.Gelu if use_gelu
            else mybir.ActivationFunctionType.Identity)

    nc = bacc.Bacc("TRN2", target_bir_lowering=False, debug=False,
                   num_devices=NCORES)
    mm = nc.tensor.matmul
    RG = [list(range(NCORES))]

    # ---- I/O ----
    xt_d = nc.dram_tensor("xt", [128, NBLK * 256], dt16, kind="ExternalInput").ap()
    xh_d = nc.dram_tensor("xh", [128, 5 * XHS], dt16, kind="ExternalInput").ap()
    tp_d = nc.dram_tensor("tp", [128, 512], dt16, kind="ExternalInput").ap()
    aux_d = nc.dram_tensor("aux", [128, BPC * 3], dt16, kind="ExternalInput").ap()
    w1_d = nc.dram_tensor("w1", [128, KB1 * 512], dt16, kind="ExternalInput").ap()
    w2_d = nc.dram_tensor("w2", [128, KB * 512], dt16, kind="ExternalInput").ap()
    w3_d = nc.dram_tensor("w3", [128, KB * 512], dt16, kind="ExternalInput").ap()
    w2l_d = nc.dram_tensor("w2l", [128, 4 * 512], dt16, kind="ExternalInput").ap()
    w3l_d = nc.dram_tensor("w3l", [128, 4 * 512], dt16, kind="ExternalInput").ap()
    pj_d = nc.dram_tensor("pj", [128, 2 * 512], dt16, kind="ExternalInput").ap()
    b1_d = nc.dram_tensor("b1", [128, 4], dt32, kind="ExternalInput").ap()
    b2_d = nc.dram_tensor("b2", [128, 4], dt32, kind="ExternalInput").ap()
    b3_d = nc.dram_tensor("b3", [1, 512], dt16, kind="ExternalInput").ap()
    out_d = nc.dram_tensor("out", [Y0, HSH], dt32, kind="ExternalOutput").ap()

    with tile.TileContext(nc) as tc:
        with (
            tc.tile_pool(name="cst", bufs=1) as cst,
            tc.tile_pool(name="pn", bufs=3) as pnp,
            tc.tile_pool(name="osb", bufs=2) as osp,
            tc.tile_pool(name="psA", bufs=1, space="PSUM") as psA,
            tc.tile_pool(name="psB", bufs=2, space="PSUM") as psB,
            tc.tile_pool(name="dram", bufs=1, space="DRAM") as drp,
        ):
            # ================= t0: tiny latency-critical loads =============
            xh_t = cst.tile([128, 5 * XHS], dt16, tag="xh")
            nc.sync.dma_start(xh_t, xh_d)
            tp_t = cst.tile([128, 512], dt16, tag="tp")
            nc.sync.dma_start(tp_t, tp_d)
            aux_t = cst.tile([128, BPC * 3], dt16, tag="aux")
            nc.sync.dma_start(aux_t, aux_d)
            b1_t = cst.tile([128, 4], dt32, tag="b1")
            nc.scalar.dma_start(b1_t, b1_d)
            b2_t = cst.tile([128, 4], dt32, tag="b2")
            nc.scalar.dma_start(b2_t, b2_d)
            b3_t = cst.tile([1, 512], dt16, tag="b3")
            nc.scalar.dma_start(b3_t, b3_d)
            ones_t = cst.tile([128, 1], dt16, tag="ones")
            nc.vector.memset(ones_t, 1.0)
            onesr_t = cst.tile([1, 128], dt16, tag="onesr")
            nc.vector.memset(onesr_t, 1.0)
            # preload the GELU activation table off the critical path
            dum_t = cst.tile([1, 1], dt16, tag="dum")
            nc.scalar.activation(dum_t, ones_t[0:1, 0:1],
                                 GELU, bias=b1_t[0:1, 0:1])

            # ================= bulk streams (deadline order) ===============
            # gpsimd ring: xt, then its half of w1/w2/w3
            xt_t = cst.tile([128, NBLK * 256], dt16, tag="xt")
            for sp in range(4):
                w = NBLK * 256 // 4
                nc.sync.dma_start(xt_t[:, w * sp: w * (sp + 1)],
                                  xt_d[:, w * sp: w * (sp + 1)])

            # w1 fully resident: its stream drains at wire speed instead of
            # being paced by fc1's matmul consumption through a small pool.
            w1R = cst.tile([128, KB1 * 512], dt16, tag="w1R")
            for sp in range(8):
                w = KB1 * 512 // 8
                nc.sync.dma_start(w1R[:, w * sp: w * (sp + 1)],
                                  w1_d[:, w * sp: w * (sp + 1)])

            # DRAM bounce buffers for the collectives
            arA_i = drp.tile([261, NN], dt32, tag="arA_i")
            arA_o = drp.tile([261, NN], dt32, tag="arA_o", addr_space="Shared")
            g1_i = drp.tile([128, 1024], dt16, tag="g1_i")
            g1_o = drp.tile([1024, 1024], dt16, tag="g1_o", addr_space="Shared")
            g2_i = drp.tile([128, 1024], dt16, tag="g2_i")
            g2_o = drp.tile([1024, 1024], dt16, tag="g2_o", addr_space="Shared")

            # ================= stage 1: cov partials =======================
            # u_ps[ic]: UT chunk [a in 128*ic.., b]; racp: rac rows 0:3 and
            # p at partition 32 (one bank); q its own bank.
            u_ps = [psA.tile([128, 256], dt32, tag=f"acc{ic}", name=f"u{ic}")
                    for ic in range(2)]
            racp_ps = psA.tile([33, 256], dt32, tag="acc2", name="racp_ps")
            q_ps = psA.tile([1, 256], dt32, tag="acc3", name="q_ps")
            for bl in range(BPC):
                xb = xh_t[:, XHS * bl: XHS * bl + 256]
                xb1 = xh_t[:, XHS * (bl + 1): XHS * (bl + 1) + 256]
                first, last = bl == 0, bl == BPC - 1
                pt_ps = psB.tile([128, 256], dt32, tag="rot", name="pt_ps")
                mm(pt_ps, tp_t[:, 0:128], xb, start=True, stop=False)
                mm(pt_ps, tp_t[:, 128:256], xb1, start=False, stop=True)
                pt_sb = pnp.tile([128, 256], dt16, tag="ptsb", name="pt_sb")
                nc.vector.tensor_copy(pt_sb, pt_ps)
                nt_ps = psB.tile([128, 256], dt32, tag="rot", name="nt_ps")
                mm(nt_ps, tp_t[:, 256:384], xb, start=True, stop=False)
                mm(nt_ps, tp_t[:, 384:512], xb1, start=False, stop=True)
                nt_sb = pnp.tile([128, 256], dt16, tag="ntsb", name="nt_sb")
                nc.vector.tensor_copy(nt_sb, nt_ps)
                for ic in range(2):
                    xbc = xh_t[:, XHS * bl + 128 * ic: XHS * bl + 128 * ic + 128]
                    mm(u_ps[ic], pt_sb[:, 128 * ic:128 * ic + 128], xb,
                       start=first, stop=False)
                    mm(u_ps[ic], xbc, nt_sb, start=False, stop=last)
                # reduction vectors: rac rows 0:3 (start clears the bank),
                # p at partition 32 rides the same bank with start=False.
                mm(racp_ps[0:3, :], aux_t[:, 3 * bl:3 * bl + 3], xb,
                   start=first, stop=False, skip_group_check=True)
                mm(racp_ps[32:33, :], ones_t, pt_sb,
                   start=False, stop=last, skip_group_check=True)
                mm(q_ps, ones_t, nt_sb, start=first, stop=last)

            # ---- stage AR input (scalar ring = empty -> lands immediately)
            stgs = []
            for ic in range(2):
                stg = cst.tile([128, 256], dt32, tag=f"stg{ic}", name=f"stg{ic}")
                nc.vector.tensor_scalar_mul(stg, u_ps[ic], 1.0 / DD)
                nc.scalar.dma_start(arA_i[128 * ic:128 * ic + 128, :], stg)
                stgs.append(stg)
            rpq_sb = cst.tile([65, 256], dt32, tag="rpq_sb")
            nc.vector.tensor_copy(rpq_sb[0:3, :], racp_ps[0:3, :])
            nc.vector.tensor_copy(rpq_sb[32:33, :], racp_ps[32:33, :])
            nc.vector.tensor_copy(rpq_sb[64:65, :], q_ps)
            nc.scalar.dma_start(arA_i[256:259, :], rpq_sb[0:3, :])
            nc.scalar.dma_start(arA_i[259:260, :], rpq_sb[32:33, :])
            nc.scalar.dma_start(arA_i[260:261, :], rpq_sb[64:65, :])
            nc.gpsimd.collective_compute(
                "AllReduce", mybir.AluOpType.add, replica_groups=RG,
                ins=[arA_i.opt()], outs=[arA_o.opt()])

            # ================= fc1 X-half (overlaps AllReduce) =============
            f1_ps = [psA.tile([128, 256], dt32, tag=f"acc{hh}", name=f"f1_{hh}")
                     for hh in range(4)]
            for k in range(NBLK):
                for hh in range(4):
                    mm(f1_ps[hh],
                       w1R[:, 512 * k + 128 * hh: 512 * k + 128 * hh + 128],
                       xt_t[:, 256 * k:256 * k + 256],
                       start=(k == 0), stop=False)

            # ================= G^T = X @ W1c (also in AR shadow) ===========
            g_ps = [psA.tile([128, 512], dt32, tag=f"acc{4 + ib}", name=f"g_{ib}")
                    for ib in range(2)]
            for k in range(NBLK):
                for ib in range(2):
                    mm(g_ps[ib],
                       xt_t[:, 256 * k + 128 * ib: 256 * k + 128 * ib + 128],
                       w1R[:, 512 * (NBLK + k): 512 * (NBLK + k) + 512],
                       start=(k == 0), stop=(k == NBLK - 1))

            # ================= remaining bulk (w2, w3, pj) =================
            # w2l/w3l early (local-shard matmuls run in the AllGather
            # shadows).  w2R/w3R/pj DMAs are emitted later behind gate-reader
            # ops with REAL data deps so the scheduler cannot hoist them into
            # the shared HWDGE FIFO ahead of the AR staging/readback.
            w2l_t = cst.tile([128, 4 * 512], dt16, tag="w2l")
            nc.sync.dma_start(w2l_t, w2l_d)
            w3l_t = cst.tile([128, 4 * 512], dt16, tag="w3l")
            nc.sync.dma_start(w3l_t, w3l_d)
            w2R = cst.tile([128, KB * 512], dt16, tag="w2R")
            w3R = cst.tile([128, KB * 512], dt16, tag="w3R")
            pj_t = cst.tile([128, 2 * 512], dt16, tag="pj")
            nc.vector.memset(w2R[0:1, 0:KB * 512:KB * 64], 0.0)
            nc.vector.memset(w3R[0:1, 0:KB * 512:KB * 64], 0.0)
            nc.vector.memset(pj_t[0:1, 0:1024:512], 0.0)
            gsc1 = cst.tile([1, 8], dt16, tag="gsc1")
            gsc2 = cst.tile([1, 8], dt16, tag="gsc2")
            gsc3 = cst.tile([1, 2], dt16, tag="gsc3")

            # ================= corrections + covT ==========================
            ured = [cst.tile([128, 256], dt32, tag=f"stg{ic}", name=f"ured{ic}")
                    for ic in range(2)]
            for ic in range(2):
                nc.scalar.dma_start(ured[ic], arA_o[128 * ic:128 * ic + 128, :])
            rows5 = cst.tile([1, 5 * 256], dt32, tag="rows5")
            nc.scalar.dma_start(rows5, arA_o[256:261, :])
            r_row, a_row, c_row = (rows5[:, 0:256], rows5[:, 256:512],
                                   rows5[:, 512:768])
            p_row, q_row = rows5[:, 768:1024], rows5[:, 1024:1280]
            # staged U (hence ured) is pre-divided by D, so alpha/beta are
            # divided by D here too; m = r/D.
            m16 = cst.tile([1, NN], dt16, tag="m16")
            nc.vector.tensor_scalar_mul(m16, r_row, 1.0 / DD)
            t_aq = cst.tile([1, NN], dt32, tag="t_aq")
            nc.vector.tensor_add(t_aq, a_row, q_row)
            al32 = cst.tile([1, NN], dt32, tag="al32")
            nc.vector.tensor_scalar_mul(al32, t_aq, 1.0 / DD)
            gm32 = cst.tile([1, NN], dt32, tag="gm32")
            nc.vector.tensor_scalar_mul(gm32, r_row, gamma / (DD * DD))
            al16 = cst.tile([1, NN], dt16, tag="al16")
            nc.vector.tensor_sub(al16, al32, gm32)
            t_pc = cst.tile([1, NN], dt32, tag="t_pc")
            nc.vector.tensor_add(t_pc, p_row, c_row)
            be16 = cst.tile([1, NN], dt16, tag="be16")
            nc.vector.tensor_scalar_mul(be16, t_pc, 1.0 / DD)

            # gate: w2R DMAs WAR-wait this reader, entering the DMA FIFO
            # only once the AR result (m16) is back.
            nc.vector.tensor_add(gsc1, m16[0:1, 0:8],
                                 w2R[0:1, 0:KB * 512:KB * 64])
            for sp in range(8):
                w = KB * 512 // 8
                nc.sync.dma_start(w2R[:, w * sp: w * (sp + 1)],
                                  w2_d[:, w * sp: w * (sp + 1)])

            covt = cst.tile([128, 2 * 256], dt16, tag="covt")
            for ic in range(2):
                corr = psB.tile([128, 256], dt32, tag="rot", name="corr")
                mm(corr, m16[:, 128 * ic:128 * ic + 128], al16,
                   start=True, stop=False)
                mm(corr, be16[:, 128 * ic:128 * ic + 128], m16,
                   start=False, stop=True)
                nc.vector.tensor_sub(covt[:, 256 * ic:256 * ic + 256],
                                     ured[ic], corr)

            # ================= fc1 cov contribution + gelu =================
            gT = cst.tile([128, 2 * 512], dt16, tag="gT")
            for ib in range(2):
                nc.vector.tensor_copy(gT[:, 512 * ib:512 * ib + 512], g_ps[ib])
            for hh in range(4):
                for ib in range(2):
                    mm(f1_ps[hh], gT[:, 512 * ib + 128 * hh: 512 * ib + 128 * hh + 128],
                       covt[:, 256 * ib:256 * ib + 256],
                       start=False, stop=(ib == 1))
            a1loc = cst.tile([128, 4 * 256], dt16, tag="a1loc")
            for hh in range(4):
                nc.scalar.activation(a1loc[:, 256 * hh:256 * hh + 256],
                                     f1_ps[hh], GELU, bias=b1_t[:, hh:hh + 1])
            nc.scalar.dma_start(g1_i, a1loc)

            # ================= AllGather a1 + fc2 ==========================
            nc.gpsimd.collective_compute(
                "AllGather", mybir.AluOpType.bypass, replica_groups=RG,
                ins=[g1_i.opt()], outs=[g1_o.opt()])
            f2_ps = [psA.tile([128, 256], dt32, tag=f"acc{hh}", name=f"f2_{hh}")
                     for hh in range(4)]
            # local chunk contribution (runs during the AllGather); the
            # corresponding k-blocks of w2R are zeroed host-side
            for j in range(BPC):
                for hh in range(4):
                    mm(f2_ps[hh],
                       w2l_t[:, 512 * j + 128 * hh: 512 * j + 128 * hh + 128],
                       a1loc[:, 256 * j:256 * j + 256],
                       start=(j == 0), stop=False)
            a1f = cst.tile([128, NBLK * 256], dt16, tag="a1f")
            for r in range(NCORES):
                eng = nc.sync if r % 2 == 0 else nc.scalar
                eng.dma_start(a1f[:, 1024 * r:1024 * r + 1024],
                              g1_o[128 * r:128 * r + 128, :])
            # gate w3R/pj on the first gathered a1 chunk: they enter the DMA
            # FIFO after the AG1 readback, clearing the wire for the AR/AG1
            # critical path while still landing well before fc3/proj.
            nc.vector.tensor_add(gsc2, a1f[0:1, 0:8],
                                 w3R[0:1, 0:KB * 512:KB * 64])
            nc.vector.tensor_add(gsc3, a1f[0:1, 8:10],
                                 pj_t[0:1, 0:1024:512])
            for sp in range(8):
                w = KB * 512 // 8
                nc.sync.dma_start(w3R[:, w * sp: w * (sp + 1)],
                                  w3_d[:, w * sp: w * (sp + 1)])
            nc.sync.dma_start(pj_t, pj_d)
            for k in range(KB):
                for hh in range(4):
                    mm(f2_ps[hh],
                       w2R[:, 512 * k + 128 * hh: 512 * k + 128 * hh + 128],
                       a1f[:, 256 * k:256 * k + 256],
                       start=False, stop=(k == KB - 1))
            a2loc = cst.tile([128, 4 * 256], dt16, tag="a2loc")
            for hh in range(4):
                nc.scalar.activation(a2loc[:, 256 * hh:256 * hh + 256],
                                     f2_ps[hh], GELU, bias=b2_t[:, hh:hh + 1])
            nc.scalar.dma_start(g2_i, a2loc)

            # ================= AllGather a2 + fc3 ==========================
            nc.gpsimd.collective_compute(
                "AllGather", mybir.AluOpType.bypass, replica_groups=RG,
                ins=[g2_i.opt()], outs=[g2_o.opt()])
            f3_ps = [psA.tile([128, 512], dt32, tag=f"acc{4 + ii}", name=f"f3_{ii}")
                     for ii in range(2)]
            for j in range(BPC):   # local chunk, runs during the AllGather
                for ii in range(2):
                    mm(f3_ps[ii],
                       a2loc[:, 256 * j + 128 * ii: 256 * j + 128 * ii + 128],
                       w3l_t[:, 512 * j: 512 * j + 512],
                       start=(j == 0), stop=False)
            a2f = cst.tile([128, NBLK * 256], dt16, tag="a1f", name="a2f")
            for r in range(NCORES):
                eng = nc.sync if r % 2 == 0 else nc.scalar
                eng.dma_start(a2f[:, 1024 * r:1024 * r + 1024],
                              g2_o[128 * r:128 * r + 128, :])
            # k outer / ii inner: consecutive matmuls alternate PSUM banks
            for k in range(KB):
                for ii in range(2):
                    mm(f3_ps[ii],
                       a2f[:, 256 * k + 128 * ii: 256 * k + 128 * ii + 128],
                       w3R[:, 512 * k: 512 * k + 512],
                       start=False, stop=False)
            o3_t = cst.tile([128, 2 * 512], dt16, tag="o3")
            for ii in range(2):
                mm(f3_ps[ii], onesr_t, b3_t, start=False, stop=True)
                nc.vector.tensor_copy(o3_t[:, 512 * ii:512 * ii + 512], f3_ps[ii])

            # ================= proj ========================================
            for pp in range(4):
                po = psB.tile([128, 512], dt32, tag="rot", name="po")
                for ii in range(2):
                    mm(po, pj_t[:, 512 * ii + 128 * pp: 512 * ii + 128 * pp + 128],
                       o3_t[:, 512 * ii:512 * ii + 512],
                       start=(ii == 0), stop=(ii == 1))
                osb = osp.tile([128, 512], dt32, tag="osb", name=f"osb{pp}")
                nc.vector.tensor_copy(osb, po)
                eng = nc.scalar if pp % 2 == 0 else nc.sync
                eng.dma_start(out_d[128 * pp:128 * pp + 128, :], osb)

    nc.compile()
    return nc


# ----------------------------------------------------------------------------
# host-side sharding / packing
# ----------------------------------------------------------------------------
def prep_in_maps(inputs):
    X = np.asarray(inputs["X"], np.float32)
    A0, A1, B0, B1, g_p, g_n, gamma = _filters(inputs)

    XT = np.ascontiguousarray(X.T)                      # [D, N]
    xt = XT.reshape(NBLK, 128, NN).transpose(1, 0, 2).reshape(128, NBLK * 256)
    xt = xt.astype(bf16)
    tp = np.concatenate([A0, A1, B0, B1], axis=1).astype(bf16)
    pjT = np.asarray(inputs["proj"], np.float64).T      # [256, 512]
    pj = pjT.reshape(2, 128, 512).transpose(1, 0, 2).reshape(128, 1024).astype(bf16)

    f64 = lambda k: np.asarray(inputs[k], np.float64)
    fc_wT = {1: f64("fc1_w").T, 2: f64("fc2_w").T, 3: f64("fc3_w").T}

    XTz = np.concatenate([XT, np.zeros((128, NN), np.float32)], axis=0)

    in_maps = []
    for c in range(NCORES):
        # xh: 5 blocks (4 local + halo), stride 264, ones col at 256
        xh = np.zeros((128, 5 * XHS), np.float32)
        for bl in range(5):
            gb = 4 * c + bl
            xh[:, XHS * bl: XHS * bl + 256] = XTz[128 * gb:128 * gb + 128]
            xh[:, XHS * bl + 256] = 1.0
        aux = np.zeros((128, BPC * 3), np.float32)
        for bl in range(BPC):
            gb = 4 * c + bl
            aux[:, 3 * bl + 0] = 1.0
            aux[:, 3 * bl + 1] = g_p[128 * gb:128 * gb + 128]
            aux[:, 3 * bl + 2] = g_n[128 * gb:128 * gb + 128]
        hs = slice(HSH * c, HSH * (c + 1))
        w1 = fc_wT[1][:, hs].reshape(KB1, 128, HSH).transpose(1, 0, 2) \
            .reshape(128, KB1 * HSH).astype(bf16)
        w2full = fc_wT[2][:, hs].reshape(KB, 128, HSH)
        w3full = fc_wT[3][:, hs].reshape(KB, 128, HSH)
        lb = slice(BPC * c, BPC * (c + 1))       # this core's local k-blocks
        w2l = w2full[lb].transpose(1, 0, 2).reshape(128, BPC * HSH).astype(bf16)
        w3l = w3full[lb].transpose(1, 0, 2).reshape(128, BPC * HSH).astype(bf16)
        w2full = w2full.copy(); w2full[lb] = 0.0
        w3full = w3full.copy(); w3full[lb] = 0.0
        w2 = w2full.transpose(1, 0, 2).reshape(128, KB * HSH).astype(bf16)
        w3 = w3full.transpose(1, 0, 2).reshape(128, KB * HSH).astype(bf16)
        b1 = f64("fc1_b")[hs].reshape(4, 128).T.astype(np.float32)
        b2 = f64("fc2_b")[hs].reshape(4, 128).T.astype(np.float32)
        b3 = f64("fc3_b")[hs].reshape(1, HSH).astype(bf16)
        in_maps.append({
            "xt": xt, "xh": xh.astype(bf16), "tp": tp,
            "aux": aux.astype(bf16), "w1": w1, "w2": w2, "w3": w3,
            "w2l": w2l, "w3l": w3l,
            "pj": pj, "b1": b1, "b2": b2, "b3": b3,
        })
    return in_maps, gamma


def run(inputs, trace=False, **kw):
    in_maps, gamma = prep_in_maps(inputs)
    key = ("nc", float(gamma))
    if key not in _CACHE:
        _CACHE[key] = build(gamma)
    nc = _CACHE[key]
    from concourse import bass_utils
    res = bass_utils.run_bass_kernel_spmd(nc, in_maps,
                                          core_ids=list(range(NCORES)),
                                          trace=trace, **kw)
    out = np.concatenate([res.results[c]["out"] for c in range(NCORES)], axis=1)
    return out.astype(np.float32), res


def kernel(**inputs) -> np.ndarray:
    out, _ = run(inputs)
    return out


if __name__ == "__main__":
    data = np.load("inputs.npz")
    inputs = {k: data[k] for k in data.files}
    expected = np.load("expected.npy")
    out = kernel(**inputs)
    scale = np.abs(expected).max()
    err = np.abs(out - expected).max() / scale
    print(f"Relative error: {err:.3e}")


# revision 22
# speedup vs baseline: 1.0075x; 1.0050x over previous
"""Trainium2 Bass kernel for nn_Model1 (lag-weighted long-run covariance + MLP).

Math: the 129-lag weighted covariance collapses algebraically:
    sum_l w_l * (Xc @ Y_l.T) = Xc @ (sum_l w_l Y_l).T
where Y_l is the (masked) l-shift of Xc.  So cov = (Xc @ P.T + N @ Xc.T)/d
with P, N two 65-tap causal FIR filters of Xc.  Centering is pushed through
the filters as rank-1 corrections so all GEMMs run on UNCENTERED X:
    cov.T * d = U.T - m (x) alpha - beta (x) m
with U = X@P0.T + N0@X.T (P0,N0 = filters of raw X), m = row means, and
alpha/beta derived from 5 reduction vectors (r,a,c,p,q) that are computed
as extra tiny matmuls and ride along in the AllReduce.

Distribution (8 cores):
  - cov stage: shard time axis (512 cols/core), one AllReduce of [261,256] f32
  - MLP: tensor-parallel over hidden (512/core), AllGather (bf16) between
    fc1->fc2 and fc2->fc3; fc3 emits batch-major so proj shards the output
    columns; final gather is a host-side concat.

Scheduling (the perf-critical part):
  - DMA rings: sync + gpsimd carry ONLY bulk weight/X streams (deadline
    order: w1 -> w2 -> w3 -> pj); scalar carries ONLY latency-critical
    small transfers (AR staging/readback, g1/g2 stores); vector+scalar
    share the gathered-activation readbacks.  This keeps the cov AllReduce
    staging from queuing behind megabytes of weights (the baseline's main
    stall: AR fired at 85us instead of ~15us).
  - Collective triggers stay on gpsimd (only engine with collective_compute)
    AFTER all its bulk dma descriptor-gens.
  - fc3 loops k-outer/ii-inner so consecutive matmuls alternate PSUM banks.
  - a1f/a2f gathered readbacks are 8 x 256KB HWDGE transfers (vector+scalar),
    not serial gpsimd DIRECT2D copies.
  - GELU table preloaded with a dummy activation at t=0.
All heavy GEMMs use bf16 operands with fp32 PSUM accumulation.
"""
import math
import numpy as np
import ml_dtypes

NCORES = 8
Q = 64
NN = 256          # n (batch/rows of X)
DD = 4096         # d (time axis)
HID = 4096
Y0 = 512
HSH = HID // NCORES    # 512 hidden shard per core
NBLK = DD // 128       # 32 time blocks
BPC = NBLK // NCORES   # 4 blocks per core
KB1 = (DD + DD) // 128  # 64 fc1 contraction blocks
KB = HID // 128         # 32 fc2/fc3 contraction blocks
XHS = 264              # xh per-block col stride (256 data + 1 ones + pad)

bf16 = ml_dtypes.bfloat16

_CACHE = {}


# ----------------------------------------------------------------------------
# host-side weight-only precompute
# ----------------------------------------------------------------------------
def _erf(x):
    return np.vectorize(math.erf, otypes=[np.float64])(x)


def _gelu64(x):
    return 0.5 * x * (1.0 + _erf(x / np.sqrt(2.0)))


def _filters(inputs):
    f64 = lambda k: np.asarray(inputs[k], np.float64)
    lags = np.arange(-Q, Q + 1, dtype=np.float64)[:, None]
    h = _gelu64(lags @ f64("wn_w1") + f64("wn_b1"))
    w = (h @ f64("wn_w2") + f64("wn_b2"))[:, 0]
    wp = w[Q:]                                    # l = 0..Q
    wnv = np.concatenate([[0.0], w[:Q][::-1]])    # wnv[l] = w[Q-l], l=1..Q
    v = np.arange(128)[:, None]
    u = np.arange(128)[None, :]
    dvu = v - u
    d2 = dvu + 128
    A0 = np.where((dvu >= 0) & (dvu <= Q), wp[np.clip(dvu, 0, Q)], 0.0)
    A1 = np.where((d2 >= 0) & (d2 <= Q), wp[np.clip(d2, 0, Q)], 0.0)
    B0 = np.where((dvu >= 1) & (dvu <= Q), wnv[np.clip(dvu, 0, Q)], 0.0)
    B1 = np.where((d2 >= 1) & (d2 <= Q), wnv[np.clip(d2, 0, Q)], 0.0)
    t = np.arange(DD)
    lim = np.minimum(Q, DD - 1 - t)
    g_p = np.cumsum(wp)[lim]
    g_n = np.cumsum(wnv)[lim]
    gamma = float(g_p.sum() + g_n.sum())
    return A0, A1, B0, B1, g_p, g_n, gamma


# ----------------------------------------------------------------------------
# bass program
# ----------------------------------------------------------------------------
def build(gamma: float, use_gelu=True):
    import concourse.bacc as bacc
    import concourse.tile as tile
    import concourse.mybir as mybir

    dt32 = mybir.dt.float32
    dt16 = mybir.dt.bfloat16
    GELU = (mybir.ActivationFunctionType.Gelu if use_gelu
            else mybir.ActivationFunctionType.Identity)

    nc = bacc.Bacc("TRN2", target_bir_lowering=False, debug=False,
                   num_devices=NCORES)
    mm = nc.tensor.matmul
    RG = [list(range(NCORES))]

    # ---- I/O ----
    xt_d = nc.dram_tensor("xt", [128, NBLK * 256], dt16, kind="ExternalInput").ap()
    xh_d = nc.dram_tensor("xh", [128, 5 * XHS], dt16, kind="ExternalInput").ap()
    tp_d = nc.dram_tensor("tp", [128, 512], dt16, kind="ExternalInput").ap()
    aux_d = nc.dram_tensor("aux", [128, BPC * 3], dt16, kind="ExternalInput").ap()
    w1_d = nc.dram_tensor("w1", [128, KB1 * 512], dt16, kind="ExternalInput").ap()
    w2_d = nc.dram_tensor("w2", [128, KB * 512], dt16, kind="ExternalInput").ap()
    w3_d = nc.dram_tensor("w3", [128, KB * 512], dt16, kind="ExternalInput").ap()
    w2l_d = nc.dram_tensor("w2l", [128, 4 * 512], dt16, kind="ExternalInput").ap()
    w3l_d = nc.dram_tensor("w3l", [128, 4 * 512], dt16, kind="ExternalInput").ap()
    pj_d = nc.dram_tensor("pj", [128, 2 * 512], dt16, kind="ExternalInput").ap()
    b1_d = nc.dram_tensor("b1", [128, 4], dt32, kind="ExternalInput").ap()
    b2_d = nc.dram_tensor("b2", [128, 4], dt32, kind="ExternalInput").ap()
    b3_d = nc.dram_tensor("b3", [1, 512], dt16, kind="ExternalInput").ap()
    out_d = nc.dram_tensor("out", [Y0, HSH], dt32, kind="ExternalOutput").ap()

    with tile.TileContext(nc) as tc:
        with (
            tc.tile_pool(name="cst", bufs=1) as cst,
            tc.tile_pool(name="pn", bufs=3) as pnp,
            tc.tile_pool(name="osb", bufs=2) as osp,
            tc.tile_pool(name="psA", bufs=1, space="PSUM") as psA,
            tc.tile_pool(name="psB", bufs=2, space="PSUM") as psB,
            tc.tile_pool(name="dram", bufs=1, space="DRAM") as drp,
        ):
            # ================= t0: tiny latency-critical loads =============
            xh_t = cst.tile([128, 5 * XHS], dt16, tag="xh")
            nc.sync.dma_start(xh_t, xh_d)
            tp_t = cst.tile([128, 512], dt16, tag="tp")
            nc.sync.dma_start(tp_t, tp_d)
            aux_t = cst.tile([128, BPC * 3], dt16, tag="aux")
            nc.sync.dma_start(aux_t, aux_d)
            b1_t = cst.tile([128, 4], dt32, tag="b1")
            nc.scalar.dma_start(b1_t, b1_d)
            b2_t = cst.tile([128, 4], dt32, tag="b2")
            nc.scalar.dma_start(b2_t, b2_d)
            b3_t = cst.tile([1, 512], dt16, tag="b3")
            nc.scalar.dma_start(b3_t, b3_d)
            ones_t = cst.tile([128, 1], dt16, tag="ones")
            nc.vector.memset(ones_t, 1.0)
            onesr_t = cst.tile([1, 128], dt16, tag="onesr")
            nc.vector.memset(onesr_t, 1.0)
            # preload the GELU activation table off the critical path
            dum_t = cst.tile([1, 1], dt16, tag="dum")
            nc.scalar.activation(dum_t, ones_t[0:1, 0:1],
                                 GELU, bias=b1_t[0:1, 0:1])

            # ================= bulk streams (deadline order) ===============
            # gpsimd ring: xt, then its half of w1/w2/w3
            xt_t = cst.tile([128, NBLK * 256], dt16, tag="xt")
            for sp in range(4):
                w = NBLK * 256 // 4
                nc.sync.dma_start(xt_t[:, w * sp: w * (sp + 1)],
                                  xt_d[:, w * sp: w * (sp + 1)])

            # w1 fully resident: its stream drains at wire speed instead of
            # being paced by fc1's matmul consumption through a small pool.
            w1R = cst.tile([128, KB1 * 512], dt16, tag="w1R")
            for sp in range(8):
                w = KB1 * 512 // 8
                nc.sync.dma_start(w1R[:, w * sp: w * (sp + 1)],
                                  w1_d[:, w * sp: w * (sp + 1)])

            # DRAM bounce buffers for the collectives
            arA_i = drp.tile([261, NN], dt32, tag="arA_i")
            arA_o = drp.tile([261, NN], dt32, tag="arA_o", addr_space="Shared")
            g1_i = drp.tile([128, 1024], dt16, tag="g1_i")
            g1_o = drp.tile([1024, 1024], dt16, tag="g1_o", addr_space="Shared")
            g2_i = drp.tile([128, 1024], dt16, tag="g2_i")
            g2_o = drp.tile([1024, 1024], dt16, tag="g2_o", addr_space="Shared")

            # ================= stage 1: cov partials =======================
            # u_ps[ic]: UT chunk [a in 128*ic.., b]; racp: rac rows 0:3 and
            # p at partition 32 (one bank); q its own bank.
            u_ps = [psA.tile([128, 256], dt32, tag=f"acc{ic}", name=f"u{ic}")
                    for ic in range(2)]
            racp_ps = psA.tile([33, 256], dt32, tag="acc2", name="racp_ps")
            q_ps = psA.tile([1, 256], dt32, tag="acc3", name="q_ps")
            for bl in range(BPC):
                xb = xh_t[:, XHS * bl: XHS * bl + 256]
                xb1 = xh_t[:, XHS * (bl + 1): XHS * (bl + 1) + 256]
                first, last = bl == 0, bl == BPC - 1
                pt_ps = psB.tile([128, 256], dt32, tag="rot", name="pt_ps")
                mm(pt_ps, tp_t[:, 0:128], xb, start=True, stop=False)
                mm(pt_ps, tp_t[:, 128:256], xb1, start=False, stop=True)
                pt_sb = pnp.tile([128, 256], dt16, tag="ptsb", name="pt_sb")
                nc.vector.tensor_copy(pt_sb, pt_ps)
                nt_ps = psB.tile([128, 256], dt32, tag="rot", name="nt_ps")
                mm(nt_ps, tp_t[:, 256:384], xb, start=True, stop=False)
                mm(nt_ps, tp_t[:, 384:512], xb1, start=False, stop=True)
                nt_sb = pnp.tile([128, 256], dt16, tag="ntsb", name="nt_sb")
                nc.vector.tensor_copy(nt_sb, nt_ps)
                for ic in range(2):
                    xbc = xh_t[:, XHS * bl + 128 * ic: XHS * bl + 128 * ic + 128]
                    mm(u_ps[ic], pt_sb[:, 128 * ic:128 * ic + 128], xb,
                       start=first, stop=False)
                    mm(u_ps[ic], xbc, nt_sb, start=False, stop=last)
                # reduction vectors: rac rows 0:3 (start clears the bank),
                # p at partition 32 rides the same bank with start=False.
                mm(racp_ps[0:3, :], aux_t[:, 3 * bl:3 * bl + 3], xb,
                   start=first, stop=False, skip_group_check=True)
                mm(racp_ps[32:33, :], ones_t, pt_sb,
                   start=False, stop=last, skip_group_check=True)
                mm(q_ps, ones_t, nt_sb, start=first, stop=last)

            # ---- stage AR input (scalar ring = empty -> lands immediately)
            stgs = []
            for ic in range(2):
                stg = cst.tile([128, 256], dt32, tag=f"stg{ic}", name=f"stg{ic}")
                nc.vector.tensor_scalar_mul(stg, u_ps[ic], 1.0 / DD)
                nc.scalar.dma_start(arA_i[128 * ic:128 * ic + 128, :], stg)
                stgs.append(stg)
            rpq_sb = cst.tile([65, 256], dt32, tag="rpq_sb")
            nc.vector.tensor_copy(rpq_sb[0:3, :], racp_ps[0:3, :])
            nc.vector.tensor_copy(rpq_sb[32:33, :], racp_ps[32:33, :])
            nc.vector.tensor_copy(rpq_sb[64:65, :], q_ps)
            nc.scalar.dma_start(arA_i[256:259, :], rpq_sb[0:3, :])
            nc.scalar.dma_start(arA_i[259:260, :], rpq_sb[32:33, :])
            nc.scalar.dma_start(arA_i[260:261, :], rpq_sb[64:65, :])
            nc.gpsimd.collective_compute(
                "AllReduce", mybir.AluOpType.add, replica_groups=RG,
                ins=[arA_i.opt()], outs=[arA_o.opt()])

            # ================= fc1 X-half (overlaps AllReduce) =============
            f1_ps = [psA.tile([128, 256], dt32, tag=f"acc{hh}", name=f"f1_{hh}")
                     for hh in range(4)]
            for k in range(NBLK):
                for hh in range(4):
                    mm(f1_ps[hh],
                       w1R[:, 512 * k + 128 * hh: 512 * k + 128 * hh + 128],
                       xt_t[:, 256 * k:256 * k + 256],
                       start=(k == 0), stop=False)

            # ================= G^T = X @ W1c (also in AR shadow) ===========
            g_ps = [psA.tile([128, 512], dt32, tag=f"acc{4 + ib}", name=f"g_{ib}")
                    for ib in range(2)]
            for k in range(NBLK):
                for ib in range(2):
                    mm(g_ps[ib],
                       xt_t[:, 256 * k + 128 * ib: 256 * k + 128 * ib + 128],
                       w1R[:, 512 * (NBLK + k): 512 * (NBLK + k) + 512],
                       start=(k == 0), stop=(k == NBLK - 1))

            # ================= remaining bulk (w2, w3, pj) =================
            # w2l/w3l early (local-shard matmuls run in the AllGather
            # shadows).  w2R/w3R/pj DMAs are emitted later behind gate-reader
            # ops with REAL data deps so the scheduler cannot hoist them into
            # the shared HWDGE FIFO ahead of the AR staging/readback.
            w2l_t = cst.tile([128, 4 * 512], dt16, tag="w2l")
            nc.sync.dma_start(w2l_t, w2l_d)
            w3l_t = cst.tile([128, 4 * 512], dt16, tag="w3l")
            nc.sync.dma_start(w3l_t, w3l_d)
            w2R = cst.tile([128, KB * 512], dt16, tag="w2R")
            w3R = cst.tile([128, KB * 512], dt16, tag="w3R")
            pj_t = cst.tile([128, 2 * 512], dt16, tag="pj")
            nc.vector.memset(w2R[0:1, 0:KB * 512:KB * 64], 0.0)
            nc.vector.memset(w3R[0:1, 0:KB * 512:KB * 64], 0.0)
            nc.vector.memset(pj_t[0:1, 0:1024:512], 0.0)
            gsc1 = cst.tile([1, 8], dt16, tag="gsc1")
            gsc2 = cst.tile([1, 8], dt16, tag="gsc2")
            gsc3 = cst.tile([1, 2], dt16, tag="gsc3")

            # ================= corrections + covT ==========================
            ured = [cst.tile([128, 256], dt32, tag=f"stg{ic}", name=f"ured{ic}")
                    for ic in range(2)]
            for ic in range(2):
                nc.scalar.dma_start(ured[ic], arA_o[128 * ic:128 * ic + 128, :])
            rows5 = cst.tile([1, 5 * 256], dt32, tag="rows5")
            nc.scalar.dma_start(rows5, arA_o[256:261, :])
            r_row, a_row, c_row = (rows5[:, 0:256], rows5[:, 256:512],
                                   rows5[:, 512:768])
            p_row, q_row = rows5[:, 768:1024], rows5[:, 1024:1280]
            # staged U (hence ured) is pre-divided by D, so alpha/beta are
            # divided by D here too; m = r/D.
            m16 = cst.tile([1, NN], dt16, tag="m16")
            nc.vector.tensor_scalar_mul(m16, r_row, 1.0 / DD)
            t_aq = cst.tile([1, NN], dt32, tag="t_aq")
            nc.vector.tensor_add(t_aq, a_row, q_row)
            al32 = cst.tile([1, NN], dt32, tag="al32")
            nc.vector.tensor_scalar_mul(al32, t_aq, 1.0 / DD)
            gm32 = cst.tile([1, NN], dt32, tag="gm32")
            nc.vector.tensor_scalar_mul(gm32, r_row, gamma / (DD * DD))
            al16 = cst.tile([1, NN], dt16, tag="al16")
            nc.vector.tensor_sub(al16, al32, gm32)
            t_pc = cst.tile([1, NN], dt32, tag="t_pc")
            nc.vector.tensor_add(t_pc, p_row, c_row)
            be16 = cst.tile([1, NN], dt16, tag="be16")
            nc.vector.tensor_scalar_mul(be16, t_pc, 1.0 / DD)

            # gate: w2R DMAs WAR-wait this reader, entering the DMA FIFO
            # only once the AR result (m16) is back.
            nc.vector.tensor_add(gsc1, m16[0:1, 0:8],
                                 w2R[0:1, 0:KB * 512:KB * 64])
            for sp in range(8):
                w = KB * 512 // 8
                nc.sync.dma_start(w2R[:, w * sp: w * (sp + 1)],
                                  w2_d[:, w * sp: w * (sp + 1)])

            covt = cst.tile([128, 2 * 256], dt16, tag="covt")
            for ic in range(2):
                corr = psB.tile([128, 256], dt32, tag="rot", name="corr")
                mm(corr, m16[:, 128 * ic:128 * ic + 128], al16,
                   start=True, stop=False)
                mm(corr, be16[:, 128 * ic:128 * ic + 128], m16,
                   start=False, stop=True)
                nc.vector.tensor_sub(covt[:, 256 * ic:256 * ic + 256],
                                     ured[ic], corr)

            # ================= fc1 cov contribution + gelu =================
            gT = cst.tile([128, 2 * 512], dt16, tag="gT")
            for ib in range(2):
                nc.vector.tensor_copy(gT[:, 512 * ib:512 * ib + 512], g_ps[ib])
            for hh in range(4):
                for ib in range(2):
                    mm(f1_ps[hh], gT[:, 512 * ib + 128 * hh: 512 * ib + 128 * hh + 128],
                       covt[:, 256 * ib:256 * ib + 256],
                       start=False, stop=(ib == 1))
            a1loc = cst.tile([128, 4 * 256], dt16, tag="a1loc")
            for hh in range(4):
                nc.scalar.activation(a1loc[:, 256 * hh:256 * hh + 256],
                                     f1_ps[hh], GELU, bias=b1_t[:, hh:hh + 1])
            nc.scalar.dma_start(g1_i, a1loc)

            # ================= AllGather a1 + fc2 ==========================
            nc.gpsimd.collective_compute(
                "AllGather", mybir.AluOpType.bypass, replica_groups=RG,
                ins=[g1_i.opt()], outs=[g1_o.opt()])
            f2_ps = [psA.tile([128, 256], dt32, tag=f"acc{hh}", name=f"f2_{hh}")
                     for hh in range(4)]
            # local chunk contribution (runs during the AllGather); the
            # corresponding k-blocks of w2R are zeroed host-side
            for j in range(BPC):
                for hh in range(4):
                    mm(f2_ps[hh],
                       w2l_t[:, 512 * j + 128 * hh: 512 * j + 128 * hh + 128],
                       a1loc[:, 256 * j:256 * j + 256],
                       start=(j == 0), stop=False)
            # warm-keeper: independent matmuls on resident data, gated on
            # a1loc so they become ready exactly when the AllGather window
            # opens; they keep the PE HAM clock at 8/8 through the idle.
            for j in range(44):
                wk = psB.tile([128, 512], dt32, tag="rot", name=f"wk1_{j}")
                mm(wk, a1loc[:, (j % 8) * 128:(j % 8) * 128 + 128],
                   w1R[:, (j % 64) * 512:(j % 64) * 512 + 512],
                   start=True, stop=True)
            a1f = cst.tile([128, NBLK * 256], dt16, tag="a1f")
            for r in range(NCORES):
                eng = nc.sync if r % 2 == 0 else nc.scalar
                eng.dma_start(a1f[:, 1024 * r:1024 * r + 1024],
                              g1_o[128 * r:128 * r + 128, :])
            # gate w3R/pj on the first gathered a1 chunk: they enter the DMA
            # FIFO after the AG1 readback, clearing the wire for the AR/AG1
            # critical path while still landing well before fc3/proj.
            nc.vector.tensor_add(gsc2, a1f[0:1, 0:8],
                                 w3R[0:1, 0:KB * 512:KB * 64])
            nc.vector.tensor_add(gsc3, a1f[0:1, 8:10],
                                 pj_t[0:1, 0:1024:512])
            for sp in range(8):
                w = KB * 512 // 8
                nc.sync.dma_start(w3R[:, w * sp: w * (sp + 1)],
                                  w3_d[:, w * sp: w * (sp + 1)])
            nc.sync.dma_start(pj_t, pj_d)
            for k in range(KB):
                for hh in range(4):
                    mm(f2_ps[hh],
                       w2R[:, 512 * k + 128 * hh: 512 * k + 128 * hh + 128],
                       a1f[:, 256 * k:256 * k + 256],
                       start=False, stop=(k == KB - 1))
            a2loc = cst.tile([128, 4 * 256], dt16, tag="a2loc")
            for hh in range(4):
                nc.scalar.activation(a2loc[:, 256 * hh:256 * hh + 256],
                                     f2_ps[hh], GELU, bias=b2_t[:, hh:hh + 1])
            nc.scalar.dma_start(g2_i, a2loc)

            # ================= AllGather a2 + fc3 ==========================
            nc.gpsimd.collective_compute(
                "AllGather", mybir.AluOpType.bypass, replica_groups=RG,
                ins=[g2_i.opt()], outs=[g2_o.opt()])
            f3_ps = [psA.tile([128, 512], dt32, tag=f"acc{4 + ii}", name=f"f3_{ii}")
                     for ii in range(2)]
            for j in range(BPC):   # local chunk, runs during the AllGather
                for ii in range(2):
                    mm(f3_ps[ii],
                       a2loc[:, 256 * j + 128 * ii: 256 * j + 128 * ii + 128],
                       w3l_t[:, 512 * j: 512 * j + 512],
                       start=(j == 0), stop=False)
            for j in range(44):
                wk = psB.tile([128, 512], dt32, tag="rot", name=f"wk2_{j}")
                mm(wk, a2loc[:, (j % 8) * 128:(j % 8) * 128 + 128],
                   w1R[:, (j % 64) * 512:(j % 64) * 512 + 512],
                   start=True, stop=True)
            a2f = cst.tile([128, NBLK * 256], dt16, tag="a1f", name="a2f")
            for r in range(NCORES):
                eng = nc.sync if r % 2 == 0 else nc.scalar
                eng.dma_start(a2f[:, 1024 * r:1024 * r + 1024],
                              g2_o[128 * r:128 * r + 128, :])
            # k outer / ii inner: consecutive matmuls alternate PSUM banks
            for k in range(KB):
                for ii in range(2):
                    mm(f3_ps[ii],
                       a2f[:, 256 * k + 128 * ii: 256 * k + 128 * ii + 128],
                       w3R[:, 512 * k: 512 * k + 512],
                       start=False, stop=False)
            o3_t = cst.tile([128, 2 * 512], dt16, tag="o3")
            for ii in range(2):
                mm(f3_ps[ii], onesr_t, b3_t, start=False, stop=True)
                nc.vector.tensor_copy(o3_t[:, 512 * ii:512 * ii + 512], f3_ps[ii])

            # ================= proj ========================================
            for pp in range(4):
                po = psB.tile([128, 512], dt32, tag="rot", name="po")
                for ii in range(2):
                    mm(po, pj_t[:, 512 * ii + 128 * pp: 512 * ii + 128 * pp + 128],
                       o3_t[:, 512 * ii:512 * ii + 512],
                       start=(ii == 0), stop=(ii == 1))
                osb = osp.tile([128, 512], dt32, tag="osb", name=f"osb{pp}")
                nc.vector.tensor_copy(osb, po)
                eng = nc.scalar if pp % 2 == 0 else nc.sync
                eng.dma_start(out_d[128 * pp:128 * pp + 128, :], osb)

    nc.compile()
    return nc


# ----------------------------------------------------------------------------
# host-side sharding / packing
# ----------------------------------------------------------------------------
def prep_in_maps(inputs):
    X = np.asarray(inputs["X"], np.float32)
    A0, A1, B0, B1, g_p, g_n, gamma = _filters(inputs)

    XT = np.ascontiguousarray(X.T)                      # [D, N]
    xt = XT.reshape(NBLK, 128, NN).transpose(1, 0, 2).reshape(128, NBLK * 256)
    xt = xt.astype(bf16)
    tp = np.concatenate([A0, A1, B0, B1], axis=1).astype(bf16)
    pjT = np.asarray(inputs["proj"], np.float64).T      # [256, 512]
    pj = pjT.reshape(2, 128, 512).transpose(1, 0, 2).reshape(128, 1024).astype(bf16)

    f64 = lambda k: np.asarray(inputs[k], np.float64)
    fc_wT = {1: f64("fc1_w").T, 2: f64("fc2_w").T, 3: f64("fc3_w").T}

    XTz = np.concatenate([XT, np.zeros((128, NN), np.float32)], axis=0)

    in_maps = []
    for c in range(NCORES):
        # xh: 5 blocks (4 local + halo), stride 264, ones col at 256
        xh = np.zeros((128, 5 * XHS), np.float32)
        for bl in range(5):
            gb = 4 * c + bl
            xh[:, XHS * bl: XHS * bl + 256] = XTz[128 * gb:128 * gb + 128]
            xh[:, XHS * bl + 256] = 1.0
        aux = np.zeros((128, BPC * 3), np.float32)
        for bl in range(BPC):
            gb = 4 * c + bl
            aux[:, 3 * bl + 0] = 1.0
            aux[:, 3 * bl + 1] = g_p[128 * gb:128 * gb + 128]
            aux[:, 3 * bl + 2] = g_n[128 * gb:128 * gb + 128]
        hs = slice(HSH * c, HSH * (c + 1))
        w1 = fc_wT[1][:, hs].reshape(KB1, 128, HSH).transpose(1, 0, 2) \
            .reshape(128, KB1 * HSH).astype(bf16)
        w2full = fc_wT[2][:, hs].reshape(KB, 128, HSH)
        w3full = fc_wT[3][:, hs].reshape(KB, 128, HSH)
        lb = slice(BPC * c, BPC * (c + 1))       # this core's local k-blocks
        w2l = w2full[lb].transpose(1, 0, 2).reshape(128, BPC * HSH).astype(bf16)
        w3l = w3full[lb].transpose(1, 0, 2).reshape(128, BPC * HSH).astype(bf16)
        w2full = w2full.copy(); w2full[lb] = 0.0
        w3full = w3full.copy(); w3full[lb] = 0.0
        w2 = w2full.transpose(1, 0, 2).reshape(128, KB * HSH).astype(bf16)
        w3 = w3full.transpose(1, 0, 2).reshape(128, KB * HSH).astype(bf16)
        b1 = f64("fc1_b")[hs].reshape(4, 128).T.astype(np.float32)
        b2 = f64("fc2_b")[hs].reshape(4, 128).T.astype(np.float32)
        b3 = f64("fc3_b")[hs].reshape(1, HSH).astype(bf16)
        in_maps.append({
            "xt": xt, "xh": xh.astype(bf16), "tp": tp,
            "aux": aux.astype(bf16), "w1": w1, "w2": w2, "w3": w3,
            "w2l": w2l, "w3l": w3l,
            "pj": pj, "b1": b1, "b2": b2, "b3": b3,
        })
    return in_maps, gamma


def run(inputs, trace=False, **kw):
    in_maps, gamma = prep_in_maps(inputs)
    key = ("nc", float(gamma))
    if key not in _CACHE:
        _CACHE[key] = build(gamma)
    nc = _CACHE[key]
    from concourse import bass_utils
    res = bass_utils.run_bass_kernel_spmd(nc, in_maps,
                                          core_ids=list(range(NCORES)),
                                          trace=trace, **kw)
    out = np.concatenate([res.results[c]["out"] for c in range(NCORES)], axis=1)
    return out.astype(np.float32), res


def kernel(**inputs) -> np.ndarray:
    out, _ = run(inputs)
    return out


if __name__ == "__main__":
    data = np.load("inputs.npz")
    inputs = {k: data[k] for k in data.files}
    expected = np.load("expected.npy")
    out = kernel(**inputs)
    scale = np.abs(expected).max()
    err = np.abs(out - expected).max() / scale
    print(f"Relative error: {err:.3e}")
